# revision 1
# baseline (speedup 1.0000x reference)
"""Trainium2 Bass kernel for nn_MixedRepeatHeads.

Computation (full shapes):
  proj[h,b,k] = einsum(x[b,d], proj_w[h,k,d]) + proj_b[h,k]
  w = mix_w[:, index]; bb = mix_b[:, index]
  decay = clip(decay_value, 0.9, 1.0) ** (1/8)
  coef[h] = w*decay (h<8) else decay
  hidden[b, h*256+k] = w[h]*proj[h,b,k] + coef[h]*cache[h,b,k] + bb[h]
  out = hidden @ out_w.T + out_b                     # [8192, 4096]

Strategy: data-parallel over batch across 8 cores (1024 rows each).
All per-head scalars are folded on the host:
  PT[d, i=h*256+k] = w[h] * proj_w[h,k,d]           # stage-A weights
  B2[i, j]         = out_w[j, i]                    # stage-B weights
  cacheF[i, b]     = coef[h]*cache[h,b,k] + (w[h]*proj_b[h,k] + bb[h])
  xT[d, b]         = x.T
Per core (b = 1024, processed in two halves of 512):
  stage A: hiddenT[i, b] = sum_d PT[d,i]^T @ xT[d,b] + cacheF[i,b]   (f32r MMs)
  stage B: outT[j, b]    = sum_i B2[i,j]^T @ hiddenT[i,b] + out_b[j] (f32r MMs)
Device output is outT [4096, 1024] per core; host transposes and concatenates.
"""

import sys

if "/opt/trn_rl_repo" not in sys.path:
    sys.path.insert(0, "/opt/trn_rl_repo")

import numpy as np

import bass_rust
import concourse.bass as bass
import concourse.tile as tile
from concourse import mybir
from concourse.bass_utils import run_bass_kernel_spmd
from concourse.vector_clock import ScopedClock

# ---------------------------------------------------------------- constants
N_HEADS = 16
HIDDEN = 256
DIM = 4096  # d == i == j == 4096
BATCH = 8192
DECAY_CONSTANT = 8
N_CORES = 8
BC = BATCH // N_CORES  # 1024 batch rows per core
HALF = BC // 2  # 512
P = 128
DT = DIM // P  # 32 tiles along any 4096 dim

F32 = mybir.dt.float32
F32R = mybir.dt.float32r

# ------------------------------------------------- walrus wait legalization
# This walrus build supports only ONE sync-wait command per instruction.
MAXW = 1


class SafeTileContext(tile.TileContext):
    def _split_waits_in_ordered(self, ordered):
        nc = self.nc
        for _bb_name, insts in ordered.items():
            new_list = []
            changed = False
            for inst in insts:
                si = inst.sync_info
                if si is not None and len(si.on_wait) > MAXW:
                    waits = list(si.on_wait)
                    ups = list(si.on_update)
                    head, tail = waits[:-MAXW], waits[-MAXW:]
                    for w in head:
                        nop = mybir.InstNoOp(
                            name=nc.get_next_instruction_name(),
                            engine=inst.engine,
                            ins=[],
                            outs=[],
                            sync_info=bass_rust.SyncInfo(on_wait=[w], on_update=[]),
                            bass_nofuse=True,
                        )
                        nc.register_instruction(nop, overwrite=True)
                        new_list.append(nop)
                    inst.sync_info = bass_rust.SyncInfo(on_wait=tail, on_update=ups)
                    changed = True
                new_list.append(inst)
            if changed:
                insts[:] = new_list
        return ordered

    def _lower_ordered_insts(self, ordered):
        self._split_waits_in_ordered(ordered)
        return super()._lower_ordered_insts(ordered)

    def _drain_and_barrier(self, tick_clock, wait_clock):
        probe = self.nc.sync.nop(nofuse=True)
        wait_clock.add_sem_waits(
            probe.ins, ScopedClock({None: tick_clock.global_clock})
        )
        si = probe.ins.sync_info
        waits = list(si.on_wait) if si is not None else []
        upd = list(si.on_update) if si is not None else []
        probe.ins.sync_info = bass_rust.SyncInfo(on_wait=waits[:MAXW], on_update=upd)
        for i in range(MAXW, len(waits), MAXW):
            n = self.nc.sync.nop(nofuse=True)
            n.ins.sync_info = bass_rust.SyncInfo(
                on_wait=waits[i : i + MAXW], on_update=[]
            )

        self.nc.sync.drain()

        self.nc.all_engine_barrier()
        assert self.sems is not None
        popped = self.nc._tile_sem_poison_stack.pop()
        assert popped is self._sem_poison
        self.nc.clear_and_free_semaphores(list(self.sems.allocated().values()))
        self.nc.all_engine_barrier()


# ------------------------------------------------------------ kernel build
def build_kernel():
    """Per-core program. DRAM params:
      xT  [DIM, BC]  f32r : x.T slice for this core
      cf  [DIM, BC]  f32  : cacheF slice (cache term + bias, i-major)
      pt  [DIM, DIM] f32r : PT  (stage-A weights, [d, i])
      b2  [DIM, DIM] f32r : B2  (stage-B weights, [i, j])
      ob  [P, DT]    f32  : out_b strided per partition: ob[p, jt] = out_b[jt*128+p]
      outT [DIM, BC] f32  : output, transposed (j-major)
    """
    nc = bass.Bass()
    xT = nc.declare_dram_parameter("xT", [DIM, BC], F32R, isOutput=False)
    cf = nc.declare_dram_parameter("cf", [DIM, BC], F32, isOutput=False)
    pt = nc.declare_dram_parameter("pt", [DIM, DIM], F32R, isOutput=False)
    b2 = nc.declare_dram_parameter("b2", [DIM, DIM], F32R, isOutput=False)
    ob = nc.declare_dram_parameter("ob", [P, DT], F32, isOutput=False)
    outT = nc.declare_dram_parameter("outT", [DIM, BC], F32, isOutput=True)

    # [d, x] -> [p, do, x] views with d-inner on partitions
    xT_v = xT.rearrange("(o p) b -> p o b", p=P)
    pt_v = pt.rearrange("(o p) i -> p o i", p=P)
    b2_v = b2.rearrange("(o p) j -> p o j", p=P)

    with SafeTileContext(nc) as tc:
        with (
            tc.tile_pool(name="xpool", bufs=1) as xpool,
            tc.tile_pool(name="hpool", bufs=1) as hpool,
            tc.tile_pool(name="spool", bufs=3) as spool,
            tc.tile_pool(name="cfpool", bufs=2) as cfpool,
            tc.tile_pool(name="opool", bufs=3) as opool,
            tc.tile_pool(name="obpool", bufs=1) as obpool,
            tc.tile_pool(name="pspool", bufs=4, space="PSUM") as pspool,
        ):
            ob_t = obpool.tile([P, DT], F32)
            nc.sync.dma_start(ob_t[:], ob[:])

            for half in range(2):
                bsl = slice(half * HALF, (half + 1) * HALF)
                x_t = xpool.tile([P, DT, HALF], F32R, tag="x")
                nc.sync.dma_start(x_t[:], xT_v[:, :, bsl])
                h_t = hpool.tile([P, DT, HALF], F32R, tag="h")

                # stage A: hiddenT[i, b] over i-tiles
                for it in range(DT):
                    blk = spool.tile([P, DT, P], F32R, tag="wblk")
                    nc.sync.dma_start(blk[:], pt_v[:, :, it * P : (it + 1) * P])
                    ps = pspool.tile([P, HALF], F32, tag="ps")
                    for dt_ in range(DT):
                        nc.tensor.matmul(
                            ps[:],
                            blk[:, dt_, :],
                            x_t[:, dt_, :],
                            start=(dt_ == 0),
                            stop=(dt_ == DT - 1),
                        )
                    cf_t = cfpool.tile([P, HALF], F32, tag="cf")
                    nc.sync.dma_start(cf_t[:], cf[it * P : (it + 1) * P, bsl])
                    nc.vector.tensor_tensor(
                        h_t[:, it, :], ps[:], cf_t[:], mybir.AluOpType.add
                    )

                # stage B: outT[j, b] over j-tiles
                for jt in range(DT):
                    blk = spool.tile([P, DT, P], F32R, tag="wblk")
                    nc.sync.dma_start(blk[:], b2_v[:, :, jt * P : (jt + 1) * P])
                    ps = pspool.tile([P, HALF], F32, tag="ps")
                    for io in range(DT):
                        nc.tensor.matmul(
                            ps[:],
                            blk[:, io, :],
                            h_t[:, io, :],
                            start=(io == 0),
                            stop=(io == DT - 1),
                        )
                    o_t = opool.tile([P, HALF], F32, tag="o")
                    nc.vector.tensor_tensor(
                        o_t[:],
                        ps[:],
                        ob_t[:, jt : jt + 1].to_broadcast((P, HALF)),
                        mybir.AluOpType.add,
                    )
                    nc.sync.dma_start(outT[jt * P : (jt + 1) * P, bsl], o_t[:])

    return nc


# ------------------------------------------------------------- host helpers
def _host_prepare(inputs):
    x = np.asarray(inputs["x"], dtype=np.float32)
    proj_w = np.asarray(inputs["proj_w"], dtype=np.float32)
    proj_b = np.asarray(inputs["proj_b"], dtype=np.float32)
    mix_w = np.asarray(inputs["mix_w"], dtype=np.float32)
    mix_b = np.asarray(inputs["mix_b"], dtype=np.float32)
    decay_value = np.asarray(inputs["decay_value"], dtype=np.float32)
    cache = np.asarray(inputs["cache"], dtype=np.float32)
    out_w = np.asarray(inputs["out_w"], dtype=np.float32)
    out_b = np.asarray(inputs["out_b"], dtype=np.float32)
    idx = int(np.asarray(inputs["index"]))

    w = mix_w[:, idx]  # [16]
    bb = mix_b[:, idx]  # [16]
    decay = np.clip(decay_value, 0.9, 1.0) ** np.float32(1.0 / DECAY_CONSTANT)
    is_col = np.arange(N_HEADS) < (N_HEADS // 2)
    coef = np.where(is_col, w * decay, decay).astype(np.float32)  # [16]

    # PT[d, i] = w[h] * proj_w[h, k, d]
    pw = (proj_w * w[:, None, None]).reshape(DIM, DIM)  # [i, d]
    PT = np.ascontiguousarray(pw.T)  # [d, i]

    B2 = np.ascontiguousarray(out_w.T)  # [i, j]

    bias_hk = w[:, None] * proj_b + bb[:, None]  # [16, 256]
    cacheF = coef[:, None, None] * cache + bias_hk[:, None, :]  # [h, b, k]
    cacheF = np.ascontiguousarray(
        cacheF.transpose(0, 2, 1).reshape(DIM, BATCH)
    )  # [i, b]

    xT = np.ascontiguousarray(x.T)  # [d, b]

    obT = np.ascontiguousarray(out_b.reshape(DT, P).T)  # [P, DT]

    in_maps = []
    for c in range(N_CORES):
        bsl = slice(c * BC, (c + 1) * BC)
        in_maps.append(
            {
                "xT": np.ascontiguousarray(xT[:, bsl]),
                "cf": np.ascontiguousarray(cacheF[:, bsl]),
                "pt": PT,
                "b2": B2,
                "ob": obT,
            }
        )
    return in_maps


def _assemble(results):
    # results: list per core of {"outT": [DIM, BC]}
    out = np.empty((BATCH, DIM), dtype=np.float32)
    for c in range(N_CORES):
        out[c * BC : (c + 1) * BC] = results[c]["outT"].T
    return out


_NC_CACHE = None


def _get_nc():
    global _NC_CACHE
    if _NC_CACHE is None:
        _NC_CACHE = build_kernel()
    return _NC_CACHE


def kernel(**inputs) -> np.ndarray:
    in_maps = _host_prepare(inputs)
    nc = _get_nc()
    res = run_bass_kernel_spmd(nc, in_maps, list(range(N_CORES)))
    return _assemble(res.results)


if __name__ == "__main__":
    # quick self-run with random data of the right shapes
    rng = np.random.default_rng(0)
    ins = {
        "x": rng.standard_normal((BATCH, DIM), dtype=np.float32),
        "proj_w": rng.standard_normal((N_HEADS, HIDDEN, DIM), dtype=np.float32) * 0.02,
        "proj_b": rng.standard_normal((N_HEADS, HIDDEN), dtype=np.float32) * 0.02,
        "mix_w": rng.standard_normal((N_HEADS, 4096), dtype=np.float32) * 0.02 + 1.0,
        "mix_b": rng.standard_normal((N_HEADS, 4096), dtype=np.float32) * 0.02,
        "decay_value": rng.uniform(0.85, 1.05, size=(N_HEADS,)).astype(np.float32),
        "cache": rng.standard_normal((N_HEADS, BATCH, HIDDEN), dtype=np.float32),
        "out_w": rng.standard_normal((DIM, DIM), dtype=np.float32) * 0.02,
        "out_b": rng.standard_normal((DIM,), dtype=np.float32) * 0.02,
        "index": 1000,
    }
    out = kernel(**ins)
    print("out", out.shape, out.dtype, float(np.abs(out).mean()))
